# revision 47
# baseline (speedup 1.0000x reference)
"""Trainium2 Bass kernel for nn_BiLSTM: 2-layer BiLSTM (B=64,T=512,D=64,H=128) + FC.

Sharding: data-parallel over batch across 8 NeuronCores (8 samples/core).

Lean fused recurrence: one Sigmoid ACT op covers all 4 gate blocks of both
directions (g pre-acts pre-scaled x2 so tanh(g)=2*sig(2g)-1), the cell
update is 3 DVE scalar_tensor_tensor ops, tanh(c) is a second Sigmoid
(v=sig(2c), tanh=2v-1), and h/2=(v-.5)*sig_o is one STT (the factor 2 is
folded into downstream weights).  Per step: PE 8 matmuls, ACT 2 ops,
DVE 5 ops, 5 semaphore waits total.

Per-core dataflow (per layer, fwd step s paired with bwd step j=s-1):
  PSUM gate bank layout is region-major: col r*64 + g*16 + lane*8 with
  r = s%8, g in {i=0,f=1,o=2,g=3}, lane 0 = bwd (step j=s-1, i.e. bwd
  step j occupies region (j+1)%8), lane 1 = fwd (step s).  Every
  per-step operand is a contiguous AP: the sigma input is cols
  r*64..r*64+64, and the h output lands on contiguous X cols
  [16s+8, 16s+24) = [bwd h_{s-1} | fwd h_s].
  X layout [128, 16*(T+2)] bf16: fwd h_t at col 16(t+1), bwd h_t at
  16(t+1)+8; cols 0..16 stay zero so step-0 recurrence rhs is just X[0:16].
  h is stored as h/2 (the 2 folded into consuming weights).
Host: reshape y -> [8,T,64] per core, concat cores -> [64,T,64].
"""
import sys, os
sys.path.insert(0, "/opt/trn_rl_repo")
import numpy as np
import ml_dtypes

import concourse.bass as bass
from concourse import mybir
from concourse.bass_utils import run_bass_kernel_spmd

F32 = mybir.dt.float32
BF16 = mybir.dt.bfloat16
BF = ml_dtypes.bfloat16
AluOp = mybir.AluOpType
ActFn = mybir.ActivationFunctionType

H = 128
NB = 4  # rotating PSUM gate banks
BLK = {"i": 0, "f": 1, "o": 2, "g": 3}          # PSUM gate-block order
PT = {"i": 0, "f": 1, "g": 2, "o": 3}           # PyTorch row-block order


def ap_of(t, off, dims):
    base = t[:] if not isinstance(t, bass.AP) else t
    return bass.AP(tensor=base.tensor, offset=base.offset + off, ap=list(dims))


def pstride(t):
    base = t[:] if not isinstance(t, bass.AP) else t
    return base.ap[0][0]


def build_nc(T=512, serial=False, dbg=False, flat_mm=False, snap=None):
    assert T % 16 == 0
    NTOK = T * 8
    XW = 16 * (T + 2)          # unified h-storage width per layer
    nc = bass.Bass("TRN2", target_bir_lowering=False, debug=False)
    dbg_d = {}
    if dbg:
        for nm in ("dX1", "dX2"):
            dbg_d[nm] = nc.dram_tensor(nm, [128, XW], F32, kind="ExternalOutput")
    if snap is not None:
        for nm, w in (("sU", 64), ("sV", 16), ("sC", 16), ("sP", 16), ("sQ", 16),
                      ("sX", 64), ("sBank", 512)):
            dbg_d[nm] = nc.dram_tensor(nm, [128, w], F32, kind="ExternalOutput")
        for k in range(7):
            dbg_d[f"sXs{k}"] = nc.dram_tensor(f"sXs{k}", [128, 48], F32, kind="ExternalOutput")

    # ---------------- DRAM I/O ----------------
    x_d = nc.dram_tensor("x", [8, T, 64], F32, kind="ExternalInput")
    wih0 = {d: nc.dram_tensor(f"wih0{d}", [64, 512], BF16, kind="ExternalInput") for d in "fb"}
    wih1 = {d: nc.dram_tensor(f"wih1{d}", [256, 512], BF16, kind="ExternalInput") for d in "fb"}
    whh_d = {(l, d): nc.dram_tensor(f"whh{l}{d}", [128, 512], BF16, kind="ExternalInput")
             for l in (0, 1) for d in "fb"}
    bias8_d = {l: nc.dram_tensor(f"bias8_{l}", [8, 128], BF16, kind="ExternalInput")
               for l in (0, 1)}
    wfc_d = nc.dram_tensor("wfc", [256, 64], BF16, kind="ExternalInput")
    mask8_d = nc.dram_tensor("mask8_in", [8, 512], BF16, kind="ExternalInput")
    id128_d = nc.dram_tensor("id128_in", [128, 128], F32, kind="ExternalInput")
    ones_d = nc.dram_tensor("ones_in", [1, 512], F32, kind="ExternalInput")
    bfc_d = nc.dram_tensor("bfc", [1, 64], F32, kind="ExternalInput")
    y_d = nc.dram_tensor("y", [64, NTOK], F32, kind="ExternalOutput")

    # ---------------- SBUF ----------------
    sb = nc.alloc_sbuf_tensor
    x_stage = sb("x_stage", [128, 4 * T], F32)
    X0 = sb("X0", [64, NTOK], BF16)
    X = {1: sb("X1", [128, XW], BF16), 2: sb("X2", [128, XW], BF16)}
    y_s = sb("y_s", [64, NTOK], F32)

    w_ih0 = {d: sb(f"w_ih0{d}", [64, 512], BF16) for d in "fb"}
    w_ih1a = {d: sb(f"w_ih1a{d}", [128, 512], BF16) for d in "fb"}
    w_ih1b = {d: sb(f"w_ih1b{d}", [128, 512], BF16) for d in "fb"}
    w_hh = {(l, d): sb(f"w_hh{l}{d}", [128, 512], BF16) for l in (0, 1) for d in "fb"}
    b8 = {l: sb(f"b8_{l}", [8, 128], BF16) for l in (0, 1)}
    wfca = sb("wfca", [128, 64], BF16)
    wfcb = sb("wfcb", [128, 64], BF16)
    bfc = sb("bfc_s", [1, 64], F32)

    mask8 = sb("mask8", [8, 512], BF16)
    ones_fc = sb("ones_fc", [1, 512], F32)
    id128 = sb("id128", [128, 128], F32)

    U_t = [sb(f"U{i}", [128, 64], F32) for i in range(2)]
    V_t = sb("V_t", [128, 16], BF16)
    P_t = sb("P_t", [128, 16], F32)
    Q_t = sb("Q_t", [128, 16], F32)
    C_t = sb("C_t", [128, 16], F32)
    spacer8 = sb("spacer8", [128, 8], F32)

    gbank = [nc.alloc_psum_tensor(f"gb{i}", [128, 512], F32) for i in range(NB)]
    tbank = [nc.alloc_psum_tensor(f"tb{i}", [64, 512], F32) for i in range(2)]

    sem_in = nc.alloc_semaphore("sem_in")
    s_mm = nc.alloc_semaphore("s_mm")
    s_act = nc.alloc_semaphore("s_act")
    s_dve = nc.alloc_semaphore("s_dve")
    s_out = nc.alloc_semaphore("s_out")
    cnt = {"mm": 0, "act": 0, "dve": 0}
    sems = {"mm": s_mm, "act": s_act, "dve": s_dve}

    def W(eng, sem, val):
        eng.wait_ge(sem, val)

    ENG = {"mm": None, "act": None, "dve": None}

    def inc(ins, which, sem):
        ins.then_inc(sem, 1)
        cnt[which] += 1
        if which == "dve":
            # HW: DVE writes are not yet globally visible when the sem inc
            # lands; a self-wait after every DVE op closes the window
            # (empirically required for correctness).
            nc.vector.wait_ge(s_dve, cnt["dve"])
        if serial is True:
            for eng in (nc.tensor, nc.scalar, nc.vector):
                for w in ("mm", "act", "dve"):
                    eng.wait_ge(sems[w], cnt[w])
        return cnt[which]

    ENG["mm"], ENG["act"], ENG["dve"] = nc.tensor, nc.scalar, nc.vector

    # ---------------- input DMAs (sync engine queues) ----------------
    n_dma = 0

    def dma(dst, src):
        nonlocal n_dma
        nc.sync.dma_start(out=dst, in_=src).then_inc(sem_in, 16)
        n_dma += 1

    dma(x_stage[:, :], x_d[:].rearrange("b t d -> (b t d)").rearrange("(p f) -> p f", p=128))
    for d in "fb":
        dma(w_ih0[d][:, :], wih0[d][:, :])
        dma(w_ih1a[d][:, :], wih1[d][0:128, :])
        dma(w_ih1b[d][:, :], wih1[d][128:256, :])
        dma(w_hh[(0, d)][:, :], whh_d[(0, d)][:, :])
        dma(w_hh[(1, d)][:, :], whh_d[(1, d)][:, :])
    for l in (0, 1):
        dma(b8[l][:, :], bias8_d[l][:, :])
    dma(wfca[:, :], wfc_d[0:128, :])
    dma(wfcb[:, :], wfc_d[128:256, :])
    dma(bfc[:, :], bfc_d[:, :])
    dma(mask8[:, :], mask8_d[:, :])
    dma(id128[:, :], id128_d[:, :])
    dma(ones_fc[:, :], ones_d[:, :])

    nc.tensor.wait_ge(sem_in, 16 * n_dma)
    nc.vector.wait_ge(sem_in, 16 * n_dma)

    # zero the X pads: fwd h_{-1} at cols 0..8; bwd h_{-1} (= time T) at
    # col 16(T+1)+8..+16
    for Xi in (X[1], X[2]):
        ins = nc.vector.memset(Xi[:, 0:16], 0.0)
        inc(ins, "dve", s_dve)
        ins = nc.vector.memset(Xi[:, 16 * (T + 1) + 8: 16 * (T + 1) + 16], 0.0)
        inc(ins, "dve", s_dve)

    # ---------------- x transpose into X0 ----------------
    TL = T // 16          # t_low values per partition-row
    copy_done = {}
    for tlo in range(TL):
        bank = tbank[tlo % 2]
        if tlo >= 2:
            W(nc.tensor, s_act, copy_done[tlo - 2])
        ins = nc.tensor.transpose(bank[0:64, 0:128],
                                  x_stage[:, tlo * 64:(tlo + 1) * 64], id128[:, :])
        trc = inc(ins, "mm", s_mm)
        src = ap_of(bank, 0, [[pstride(bank), 64], [1, 16], [16, 8]])
        dst = ap_of(X0, tlo * 8, [[pstride(X0), 64], [TL * 8, 16], [1, 8]])
        W(nc.scalar, s_mm, trc)
        ins = nc.scalar.activation(dst, src, ActFn.Copy)
        copy_done[tlo] = inc(ins, "act", s_act)

    # ---------------- BiLSTM layers ----------------
    # region-major PSUM: col = r*64 + g*16 + lane*8, lane 0=bwd 1=fwd
    def gates_ap(bank, g, r, lane, nb=8, nr=1):
        off = r * 64 + g * 16 + lane * 8
        dims = [[pstride(bank), 128]]
        if nr > 1:
            dims.append([64, nr])
        dims.append([1, nb])
        return ap_of(bank, off, dims)

    def layer(l, parts_f, parts_b, XO):
        """parts_*: list of (lhsT_sbuf, rhs_ap_fn); rhs_ap_fn(t0, n, step)
        returns the input AP covering timesteps t0, t0+step, ... (n of them)."""
        n_chunks = T // 8 + 1
        h_done, sig_done = {}, {}

        # barrier: everything ACT/DVE emitted so far must finish before PE
        # writes gate banks / reads X sources of this layer
        W(nc.tensor, s_act, cnt["act"])
        W(nc.tensor, s_dve, cnt["dve"])
        ins = nc.vector.memset(C_t[:, :], 0.0)
        mc = inc(ins, "dve", s_dve)
        W(nc.vector, s_dve, mc)

        def pregate(c):
            if c >= n_chunks:
                return
            bank = gbank[c % NB]
            nc.tensor.matmul(bank[:, :], b8[l][:, :], mask8[:, :],
                             start=True, stop=False, skip_group_check=True)
            # fwd: region t%8 cols +0 for t in [8c, 8c+8)
            t0, t1 = 8 * c, min(8 * c + 8, T)
            if t0 < t1:
                for (lhsT, rhs_fn) in parts_f:
                    kr = lhsT.shape[0]
                    if flat_mm:
                        for k in range(t1 - t0):
                            rhs = rhs_fn(t0 + k, 1, +1)
                            for g in range(4):
                                nc.tensor.matmul(
                                    gates_ap(bank, g, k, 1, nb=8, nr=1),
                                    lhsT[0:kr, g * 128:(g + 1) * 128], rhs,
                                    start=False, stop=False, skip_group_check=True)
                    else:
                        rhs = rhs_fn(t0, t1 - t0, +1)
                        for g in range(4):
                            nc.tensor.matmul(
                                gates_ap(bank, g, 0, 1, nb=8, nr=t1 - t0),
                                lhsT[0:kr, g * 128:(g + 1) * 128], rhs,
                                start=False, stop=False, skip_group_check=True)
            # bwd: region rho holds step j = 8c-1+rho at time tt = T-8c-rho
            rho_lo = 1 if c == 0 else 0
            rho_hi = min(7, T - 8 * c)
            if rho_lo <= rho_hi:
                nr = rho_hi - rho_lo + 1
                tt_start = T - 8 * c - rho_lo
                # bwd time runs opposite to the region order; unroll
                # per-region to keep every AP stride positive.
                for (lhsT, rhs_fn) in parts_b:
                    kr = lhsT.shape[0]
                    for k in range(nr):
                        rhs = rhs_fn(tt_start - k, 1, +1)
                        for g in range(4):
                            nc.tensor.matmul(
                                gates_ap(bank, g, rho_lo + k, 0, nb=8, nr=1),
                                lhsT[0:kr, g * 128:(g + 1) * 128], rhs,
                                start=False, stop=False, skip_group_check=True)

        for c in range(min(NB, n_chunks)):
            pregate(c)

        for s in range(T + 1):
            bank = gbank[(s // 8) % NB]
            r = s % 8
            # ---- PE: recurrence matmuls (one wait covers both dirs) ----
            # h_b(j-1) lives at time T-j  ->  col 16(T-j+1)+8
            rhs_b = (XO[:, 16 * (T - s + 2) + 8: 16 * (T - s + 2) + 16]
                     if s >= 1 else None)
            rhs_f = XO[:, 16 * s: 16 * s + 8] if s <= T - 1 else None
            if s >= 1:
                W(nc.tensor, s_dve, h_done[s - 1])
            last = None
            for g in range(4):
                if rhs_b is not None:
                    last = nc.tensor.matmul(gates_ap(bank, g, r, 0),
                                            w_hh[(l, "b")][:, g * 128:(g + 1) * 128],
                                            rhs_b, start=False, stop=True,
                                            skip_group_check=True)
                if rhs_f is not None:
                    last = nc.tensor.matmul(gates_ap(bank, g, r, 1),
                                            w_hh[(l, "f")][:, g * 128:(g + 1) * 128],
                                            rhs_f, start=False, stop=True,
                                            skip_group_check=True)
            mm_here = inc(last, "mm", s_mm)

            ub = U_t[s % 2]

            def xprobe(k):
                if (snap is not None and snap[1] in (98, 99) and snap[0] == l
                        and k == 5
                        and (1 if snap[1] == 99 else 8) <= s <= (7 if snap[1] == 99 else 14)):
                    kk = s - (1 if snap[1] == 99 else 8)
                    buf = sb(f"snap_xs{kk}", [128, 48], F32)
                    nc.vector.memset(buf[:, :], 0.0)
                    ins = nc.vector.tensor_copy(buf[:, 0:24], X[1][:, 0:24])
                    cs2 = inc(ins, "dve", s_dve)
                    nc.sync.wait_ge(s_dve, cs2)
                    nc.sync.dma_start(out=dbg_d[f"sXs{kk}"][:, :], in_=buf[:, :]).then_inc(s_out, 16)

            xprobe(0)

            # ---- ACT: u = sigma(all 4 gate blocks, both lanes) ----
            W(nc.scalar, s_mm, mm_here)
            ins = nc.scalar.activation(
                ub[:, :], ap_of(bank, r * 64, [[pstride(bank), 128], [1, 64]]),
                ActFn.Sigmoid)
            sig_done[s] = inc(ins, "act", s_act)

            def u_blk(gname):
                o0 = BLK[gname] * 16
                return ub[:, o0:o0 + 16]

            # ---- DVE: cell update ----
            W(nc.vector, s_act, sig_done[s])
            xprobe(1)
            # Q = u_f * C   (C = 2c from previous step)
            ins = nc.vector.tensor_tensor(
                out=Q_t[:, :], in0=u_blk("f"), in1=C_t[:, :], op=AluOp.mult)
            inc(ins, "dve", s_dve)
            # P = (u_g - .5) * u_i
            ins = nc.vector.scalar_tensor_tensor(
                out=P_t[:, :], in0=u_blk("g"), scalar=0.5, in1=u_blk("i"),
                op0=AluOp.subtract, op1=AluOp.mult)
            inc(ins, "dve", s_dve)
            xprobe(2)
            nc.vector.memset(spacer8[:, :], 0.0)   # gap-1 guard for P
            # C' = 4P + Q.  At s=0 the bwd lane (cols 0..8) must keep its
            # zero init (it is c_b(-1) for the j=0 step), so write fwd only.
            lo = 8 if s == 0 else 0
            ins = nc.vector.scalar_tensor_tensor(
                out=C_t[:, lo:16], in0=P_t[:, lo:16], scalar=4.0,
                in1=Q_t[:, lo:16], op0=AluOp.mult, op1=AluOp.add)
            cpr = inc(ins, "dve", s_dve)
            xprobe(3)

            # ---- ACT: v = sigma(C')  (tanh(c) = 2v - 1) ----
            W(nc.scalar, s_dve, cpr)
            ins = nc.scalar.activation(V_t[:, :], C_t[:, :], ActFn.Sigmoid)
            tau = inc(ins, "act", s_act)

            # ---- DVE: h/2 = (v - .5) * u_o -> X ----
            # fwd h_s -> col 16(s+1); bwd h_{s-1} -> time T-s -> col 16(T-s+1)+8.
            # Stream order is [bwd 8 | fwd 8] (the lane order in U/C/V).
            if s == 0:
                hdst = ap_of(XO, 16, [[pstride(XO), 128], [1, 8]])
                vsrc = ap_of(V_t, 8, [[pstride(V_t), 128], [1, 8]])
                osrc = ap_of(ub, BLK["o"] * 16 + 8, [[pstride(ub), 128], [1, 8]])
            elif s == T:
                hdst = ap_of(XO, 24, [[pstride(XO), 128], [1, 8]])
                vsrc = ap_of(V_t, 0, [[pstride(V_t), 128], [1, 8]])
                osrc = ap_of(ub, BLK["o"] * 16, [[pstride(ub), 128], [1, 8]])
            else:
                b0 = 16 * (T - s + 1) + 8
                f0 = 16 * (s + 1)
                hdst = ap_of(XO, b0, [[pstride(XO), 128], [f0 - b0, 2], [1, 8]])
                vsrc = V_t[:, :]
                osrc = u_blk("o")
            W(nc.vector, s_act, tau)
            xprobe(4)
            ins = nc.vector.scalar_tensor_tensor(
                out=hdst, in0=vsrc, scalar=0.5, in1=osrc,
                op0=AluOp.subtract, op1=AluOp.mult)
            h_done[s] = inc(ins, "dve", s_dve)
            xprobe(5)

            if snap is not None and snap == (l, s):
                snapb = sb("snapb", [128, 512], F32)
                snaps = {nm: sb(f"snap_{nm}", [128, w], F32)
                         for nm, w in (("sU", 64), ("sV", 16), ("sC", 16),
                                       ("sP", 16), ("sQ", 16), ("sX", 64))}
                ins = nc.vector.tensor_copy(snapb[:, :], bank[:, :])
                cs = inc(ins, "dve", s_dve)
                nc.vector.memset(snaps["sX"][:, :], 0.0)
                for nm, buf in (("sU", ub), ("sV", V_t), ("sC", C_t),
                                ("sP", P_t), ("sQ", Q_t), ("sX", XO)):
                    w = 24 if nm == "sX" else snaps[nm].shape[1]
                    ins = nc.vector.tensor_copy(snaps[nm][:, 0:w], buf[:, 0:w])
                    cs = inc(ins, "dve", s_dve)
                nc.sync.wait_ge(s_dve, cs)
                nc.sync.dma_start(out=dbg_d["sBank"][:, :], in_=snapb[:, :]).then_inc(s_out, 16)
                for nm in snaps:
                    nc.sync.dma_start(out=dbg_d[nm][:, :], in_=snaps[nm][:, :]).then_inc(s_out, 16)
                nc.sync.wait_ge(s_out, 16 * 7)

            # ---- PE: pregate burst for chunk c+NB into the bank just freed ----
            if r == 7:
                cc = s // 8 + NB
                if cc < n_chunks:
                    W(nc.tensor, s_act, sig_done[s])
                    W(nc.tensor, s_dve, cnt["dve"])
                    pregate(cc)

    def x0_rhs(t0, n, step):
        return ap_of(X0, t0 * 8, [[pstride(X0), 64], [8 * step, n], [1, 8]])

    def x_rhs(Xt, half):
        # both halves stored time-indexed: fwd h_t at 16(t+1), bwd h_t at
        # 16(t+1)+8
        def fn(t0, n, step):
            return ap_of(Xt, 16 * (t0 + 1) + 8 * half,
                         [[pstride(Xt), 128], [16 * step, n], [1, 8]])
        return fn

    layer(0, [(w_ih0["f"], x0_rhs)], [(w_ih0["b"], x0_rhs)], X[1])
    layer(1, [(w_ih1a["f"], x_rhs(X[1], 0)), (w_ih1b["f"], x_rhs(X[1], 1))],
          [(w_ih1a["b"], x_rhs(X[1], 0)), (w_ih1b["b"], x_rhs(X[1], 1))], X[2])

    # ---------------- FC ----------------
    W(nc.tensor, s_act, cnt["act"])
    W(nc.tensor, s_dve, cnt["dve"])
    fc_copy = {}
    CW = min(512, NTOK)        # tokens (t*8+b) per FC chunk
    n_fc = NTOK // CW
    for i in range(n_fc):
        st = i * CW
        t0 = st // 8
        nt = CW // 8
        bank = tbank[i % 2]
        if i >= 2:
            eng, c0 = fc_copy[i - 2]
            W(nc.tensor, s_act if eng == "act" else s_dve, c0)
        nc.tensor.matmul(bank[0:64, 0:CW], bfc[:, :], ones_fc[:, 0:CW],
                         start=True, stop=False, skip_group_check=True)
        rf = ap_of(X[2], 16 * (t0 + 1), [[pstride(X[2]), 128], [16, nt], [1, 8]])
        rb = ap_of(X[2], 16 * (t0 + 1) + 8, [[pstride(X[2]), 128], [16, nt], [1, 8]])
        nc.tensor.matmul(bank[0:64, 0:CW], wfca[:, :], rf,
                         start=False, stop=False, skip_group_check=True)
        ins = nc.tensor.matmul(bank[0:64, 0:CW], wfcb[:, :], rb,
                               start=False, stop=True, skip_group_check=True)
        mmc = inc(ins, "mm", s_mm)
        if i % 2 == 0:
            W(nc.scalar, s_mm, mmc)
            ins = nc.scalar.activation(y_s[:, st:st + CW], bank[0:64, 0:CW],
                                       ActFn.Copy)
            fc_copy[i] = ("act", inc(ins, "act", s_act))
        else:
            W(nc.vector, s_mm, mmc)
            ins = nc.vector.tensor_copy(y_s[:, st:st + CW], bank[0:64, 0:CW])
            fc_copy[i] = ("dve", inc(ins, "dve", s_dve))

    # ---------------- output DMA ----------------
    nc.sync.wait_ge(s_act, cnt["act"])
    nc.sync.wait_ge(s_dve, cnt["dve"])
    n_out = 1 + (0 if snap is None else 7)
    nc.sync.dma_start(out=y_d[:, :], in_=y_s[:, :]).then_inc(s_out, 16)
    if dbg:
        dcast = sb("dcast", [128, XW], F32)
        nw = 16 * T + 16
        for nm, buf in (("dX1", X[1]), ("dX2", X[2])):
            ins = nc.vector.tensor_copy(dcast[:, 0:nw], buf[:, 0:nw])
            cc = inc(ins, "dve", s_dve)
            nc.sync.wait_ge(s_dve, cc)
            nc.sync.dma_start(out=dbg_d[nm][:, 0:nw], in_=dcast[:, 0:nw]).then_inc(s_out, 16)
            nc.sync.wait_ge(s_out, 16 * (n_out + 1))
            n_out += 1
    nc.sync.wait_ge(s_out, 16 * n_out)
    return nc


# ====================== host-side prep & entry point ======================

def _to_bf(a):
    return np.asarray(a, dtype=np.float32).astype(BF)


def prep_weights(inp, l, suf_f, suf_b, h_scale_in, h_scale_self):
    """Build per-layer lhsT tensors + bias8 from PyTorch-layout weights.

    h_scale_in: scale folded into w_ih (input h values are h/2).
    h_scale_self: scale folded into w_hh (own h values are h/2).
    g-gate rows are additionally scaled x2 (tanh(g) = 2*sigmoid(2g) - 1).
    """
    out = {}
    for dname, suf in (("f", suf_f), ("b", suf_b)):
        wih = np.asarray(inp[f"w_ih_l{l}{suf}"], np.float32)   # [512, Din]
        whh = np.asarray(inp[f"w_hh_l{l}{suf}"], np.float32)   # [512, 128]
        bsum = (np.asarray(inp[f"b_ih_l{l}{suf}"], np.float32)
                + np.asarray(inp[f"b_hh_l{l}{suf}"], np.float32))  # [512]
        blocks_ih, blocks_hh, bias_rows = [], [], {}
        for gname, blk in BLK.items():
            rows = slice(PT[gname] * 128, (PT[gname] + 1) * 128)
            gs = 2.0 if gname == "g" else 1.0
            blocks_ih.append((gs * h_scale_in * wih[rows]).T)       # [Din, 128]
            blocks_hh.append((gs * h_scale_self * whh[rows]).T)     # [128, 128]
            bias_rows[blk] = gs * bsum[rows]
        out[f"wih_{dname}"] = _to_bf(np.concatenate(blocks_ih, axis=1))  # [Din, 512]
        out[f"whh_{dname}"] = _to_bf(np.concatenate(blocks_hh, axis=1))  # [128, 512]
        out[f"bias_{dname}"] = bias_rows
    bias8 = np.zeros((8, 128), np.float32)
    for blk in range(4):
        bias8[blk * 2 + 0] = out["bias_f"][blk]
        bias8[blk * 2 + 1] = out["bias_b"][blk]
    out["bias8"] = _to_bf(bias8)
    return out


def _mask8_np():
    # bias8 row j = g*2 + dd with dd 0=fwd 1=bwd; PSUM col r*64 + g*16 + lane*8
    # with lane 0=bwd 1=fwd.
    m = np.zeros((8, 512), np.float32)
    for j in range(8):
        g, dd = j // 2, j % 2
        lane = 1 - dd
        for r in range(8):
            c0 = r * 64 + g * 16 + lane * 8
            m[j, c0:c0 + 8] = 1.0
    return m.astype(BF)


_NC_CACHE = {}


def _get_nc(T, serial=False, dbg=False, flat_mm=False):
    key = (T, serial, dbg, flat_mm)
    if key not in _NC_CACHE:
        _NC_CACHE[key] = build_nc(T, serial, dbg, flat_mm)
    return _NC_CACHE[key]


def run_cores(inputs, T=512, n_cores=8, trace=False, serial=False, dbg=False,
              flat_mm=False):
    x = np.asarray(inputs["x"], np.float32)
    per = 8

    # layer0 input x is raw (scale 1); stored h values are h/2
    l0 = prep_weights(inputs, 0, "", "r", 1.0, 2.0)
    l1 = prep_weights(inputs, 1, "", "r", 2.0, 2.0)
    wfc = _to_bf(2.0 * np.asarray(inputs["w_fc"], np.float32).T)   # [256, 64]
    bfc = np.asarray(inputs["b_fc"], np.float32).reshape(1, 64)

    common = {
        "wih0f": l0["wih_f"], "wih0b": l0["wih_b"],
        "wih1f": l1["wih_f"], "wih1b": l1["wih_b"],
        "whh0f": l0["whh_f"], "whh0b": l0["whh_b"],
        "whh1f": l1["whh_f"], "whh1b": l1["whh_b"],
        "bias8_0": l0["bias8"], "bias8_1": l1["bias8"],
        "wfc": wfc, "bfc": bfc,
        "mask8_in": _mask8_np(), "id128_in": np.eye(128, dtype=np.float32),
        "ones_in": np.ones((1, 512), np.float32),
    }
    in_maps = []
    for c in range(n_cores):
        m = dict(common)
        m["x"] = np.ascontiguousarray(x[c * per:(c + 1) * per, :T])
        in_maps.append(m)

    nc = _get_nc(T, serial, dbg, flat_mm)
    res = run_bass_kernel_spmd(nc, in_maps, core_ids=list(range(n_cores)),
                               trace=trace)
    outs = []
    for c in range(n_cores):
        yc = res.results[c]["y"]                      # [64, T*8]
        outs.append(yc.reshape(64, T, 8).transpose(2, 1, 0))
    return np.concatenate(outs, axis=0), res


def kernel(**inputs):
    y, _ = run_cores(inputs, T=512, n_cores=8)
    return y.astype(np.float32)
